# revision 38
# baseline (speedup 1.0000x reference)
"""Trainium2 Bass kernel for nn_Filter: per-frame FIR filtering via STFT-style
framing (frame 512, hop 256, 128-tap filters from per-frame amplitudes),
windowed overlap-add, and peak renormalization. 8 NeuronCores, pure data
parallel (4 batches/core). Host does layout transposes, the amplitude
pointwise map, transform-matrix construction, and the final normalization.

Three device paths, dispatched by the numerical structure of the filters:

1. General (per-frame filters): frame convolutions evaluated circularly at
   N=639 (=512+128-1, alias-free) as dense shared-weight matmuls batched over
   frames on the moving dim:
     F  = rfft_639(frames)      [512 -> 640 reals]  (PE)
     R  = filter real-spectrum  [66  -> 640 reals]  (PE; the impulse is
          symmetric about tap 64 so its spectrum is real after removing a
          constant phase, folded into the inverse matrix)
     S  = F * R                 elementwise         (DVE)
     out = [S_{p-1}; S_p] @ IM2 [1280 -> 256]       (PE; irfft + roll + window
          + overlap-add all folded into one matrix)
   ~100 us on hardware.

2. Frame-constant filters: the chain collapses per batch into one matrix
   C3 [768, 256] applied to overlapping 768-sample segments. C3 is banded
   (128-tap filter), so each 128-output chunk needs only 3 of the 6 K-chunks.
   ~41 us.

3. C3 numerically diagonal (flat filter magnitudes, e.g. the all-ones
   amplitudes of the spec): the operator is an elementwise periodic gain
   out[256p + c] = v[c] * x[256p + c]; pure DVE + DMA at the memory roofline.
   ~25 us.

All matmul/stream tensors are bf16 (PE streams 1 column/cycle at 2.4 GHz vs
2 cycles/column for fp32r and 4 for fp32); accumulation stays fp32 in PSUM.
"""
import math
import numpy as np

import concourse.bass as bass
import concourse.mybir as mybir
from concourse.tile import TileContext
from concourse.bass_utils import run_bass_kernel_spmd

F32 = mybir.dt.float32
# Matmul streaming dtype. fp32r measured ~2 PE-cycles/column; bf16 streams at
# full rate and enables fast weight load.
F32R = mybir.dt.bfloat16
NP_STREAM = mybir.dt.np(F32R)

B = 32                      # total batches
BPC = 4                     # batches per core
NCORES = 8
T = 262144                  # signal length
BLOCK = 512
HOP = 256
NB = 65                     # bands
FS = 128                    # filter taps
NFFT = 639
BINS = 320                  # rfft_639 complex bins; 2*BINS = 640 reals
NF = 1024                   # frames per batch that matter
ROWS = 2052                 # 128-sample signal rows (>= (2*1023+3)+1, padded)
LN10 = math.log(10.0)


class _TC(TileContext):
    pass


def _split_multi_waits(nc):
    """This walrus build allows only one sync-wait per instruction: hoist the
    extra waits onto single-wait NOPs inserted just before, on the same engine."""
    for fn in nc.m.functions:
        for bb in fn.blocks:
            insts = list(bb.instructions)
            if not any(
                i.sync_info is not None and len(i.sync_info.on_wait) > 1
                for i in insts
            ):
                continue
            new = []
            for inst in insts:
                si = inst.sync_info
                if si is not None and len(si.on_wait) > 1:
                    waits = list(si.on_wait)
                    for k, w in enumerate(waits[:-1]):
                        nop = mybir.InstNoOp(
                            name=f"{inst.name}-w{k}",
                            engine=inst.engine,
                            sync_info=mybir.SyncInfo(on_wait=[w], on_update=[]),
                        )
                        nc.register_instruction(nop, overwrite=True)
                        new.append(nop)
                    inst.sync_info = mybir.SyncInfo(
                        on_wait=[waits[-1]], on_update=list(si.on_update)
                    )
                new.append(inst)
            bb.instructions[:] = new


def _build_host_matrices():
    """WRx [66, 640], FW [512, 640], IM2 [1280, 256], all float32."""
    hannP = 0.5 * (1.0 - np.cos(2.0 * np.pi * np.arange(FS) / FS))
    winS = np.hanning(BLOCK)

    phase = np.exp(2j * np.pi * np.arange(BINS) * (FS // 2) / NFFT)
    Rhat = np.zeros((NB, BINS))
    for q in range(NB):
        e = np.zeros(NB)
        e[q] = 1.0
        imp = np.roll(np.fft.irfft(e, n=FS), FS // 2) * hannP
        spec = np.fft.rfft(imp, n=NFFT) * phase
        Rhat[q] = spec.real
    WR = np.zeros((NB + 1, BINS))
    WR[:NB] = 20.0 * Rhat
    WR[NB] = 1e-6 * Rhat.sum(axis=0)
    WRx = np.concatenate([WR, WR], axis=1)                    # [66, 640]

    ang = -2.0 * np.pi * np.arange(BLOCK)[:, None] * np.arange(BINS)[None, :] / NFFT
    FW = np.concatenate([np.cos(ang), np.sin(ang)], axis=1)   # [512, 640]

    IM = np.zeros((2 * BINS, BLOCK))
    ephase = np.exp(-2j * np.pi * np.arange(BINS) * (FS // 2) / NFFT)
    jj = (np.arange(BLOCK) + FS // 2) % NFFT
    for w in range(BINS):
        spec = np.zeros(BINS, dtype=np.complex128)
        spec[w] = ephase[w]
        IM[w] = winS * np.fft.irfft(spec, n=NFFT)[jj]
        spec[w] = 1j * ephase[w]
        IM[BINS + w] = winS * np.fft.irfft(spec, n=NFFT)[jj]
    IM2 = np.concatenate([IM[:, HOP:], IM[:, :HOP]], axis=0)  # [1280, 256]
    return WRx.astype(np.float32), FW.astype(np.float32), IM2.astype(np.float32)


def _build_nc():
    nc = bass.Bass(trn_type="TRN2")
    xt_d = nc.dram_tensor("xt", [BPC, 128, ROWS], F32R, kind="ExternalInput")
    wm_d = nc.dram_tensor("wm", [128, 640 + BPC * NF], F32R, kind="ExternalInput")
    wk_d = nc.dram_tensor("wk", [128, 5120], F32R, kind="ExternalInput")
    out_d = nc.dram_tensor("outp", [BPC, 256, NF], F32, kind="ExternalOutput")

    with _TC(nc) as tc:
        with (
            tc.tile_pool(name="const", bufs=1) as cpool,
            tc.tile_pool(name="xtp", bufs=3) as xt_pool,
            tc.tile_pool(name="sp", bufs=2) as s_pool,
            tc.tile_pool(name="rxp", bufs=2) as rx_pool,
            tc.tile_pool(name="obp", bufs=3) as ob_pool,
            tc.tile_pool(name="pf", bufs=3, space="PSUM") as pf_pool,
            tc.tile_pool(name="pr", bufs=2, space="PSUM") as pr_pool,
            tc.tile_pool(name="po", bufs=2, space="PSUM") as po_pool,
        ):
            # PE warmup: dense dummy matmuls with no DMA dependency, issued
            # while the input DMAs land, so HAM un-throttles before real work.
            warm_sb = cpool.tile([128, 128], F32R, tag="warm", name="warm_sb")
            nc.vector.memset(warm_sb[:], 0.0)
            with tc.tile_pool(name="pw", bufs=1, space="PSUM") as pw_pool:
                w_ps = pw_pool.tile([128, 128], F32, tag="w", name="w_ps")
                for _ in range(32):
                    nc.tensor.matmul(out=w_ps[:], lhsT=warm_sb[:], rhs=warm_sb[:],
                                     start=True, stop=True)

            # first signal tile ahead of everything on the sync queue: the
            # first PE work after warmup is rfft on it
            xg_first = xt_pool.tile([128, 1028], F32R, tag="xg", name="xg")
            nc.sync.dma_start(out=xg_first[:], in_=xt_d[0][:, 0:1028])
            # then the wr + first group's mag columns
            wm_sb = cpool.tile([128, 640 + BPC * NF], F32R, tag="wm", name="wm_sb")
            nc.sync.dma_start(out=wm_sb[:, 0:1152], in_=wm_d[:, 0:1152])
            # weight matrices + the rest of mag on the scalar-engine HWDGE
            # path, parallel to the sync-engine input loads; ordered by first use
            wk_sb = cpool.tile([128, 5120], F32R, tag="wk", name="wk_sb")
            nc.scalar.dma_start(out=wk_sb[:, 0:2560], in_=wk_d[:, 0:2560])
            nc.scalar.dma_start(out=wm_sb[:, 1152:640 + BPC * NF],
                                in_=wm_d[:, 1152:640 + BPC * NF])
            nc.scalar.dma_start(out=wk_sb[:, 2560:5120], in_=wk_d[:, 2560:5120])
            wr_sb = wm_sb[:, 0:640]
            mag_sb = wm_sb[:, 640:640 + BPC * NF]
            fw_blk = lambda i, m: wk_sb[:, (4 * m + i) * 128:(4 * m + i) * 128 + 128]
            im_sb = [wk_sb[:, 2560 + 256 * k:2560 + 256 * (k + 1)] for k in range(10)]

            pending = None  # deferred irfft work: (s_tiles, g, b)

            def emit_irfft(s_tiles, g, b):
                for mo in range(2):
                    o_ps = po_pool.tile([128, 512], F32, tag="o", name="o_ps")
                    for kc in range(10):
                        scol = 512 * g + (1 if kc >= 5 else 0)
                        nc.tensor.matmul(
                            out=o_ps[:],
                            lhsT=(im_sb[kc][:, 128 * mo:128 * (mo + 1)]),
                            rhs=(s_tiles[kc % 5][:, scol:scol + 512]),
                            start=(kc == 0),
                            stop=(kc == 9),
                        )
                    ob = ob_pool.tile([128, 512], F32, tag="ob", name="ob")
                    nc.scalar.copy(out=ob[:], in_=o_ps[:])
                    nc.sync.dma_start(
                        out=out_d[b, 128 * mo:128 * (mo + 1), 512 * g:512 * (g + 1)],
                        in_=ob[:],
                    )

            def emit_gmap(b, g):
                rx = []
                for m in range(5):
                    r_ps = pr_pool.tile([128, 512], F32, tag="r", name="r_ps")
                    nc.tensor.matmul(
                        out=r_ps[:],
                        lhsT=(wr_sb[:, 128 * m:128 * (m + 1)]),
                        rhs=(mag_sb[:, NF * b + 512 * g:NF * b + 512 * (g + 1)]),
                        start=True,
                        stop=True,
                    )
                    rxm = rx_pool.tile([128, 512], F32R, tag=f"rx{m}", name=f"rx{m}")
                    nc.scalar.copy(out=rxm[:], in_=r_ps[:])
                    rx.append(rxm)
                return rx

            def emit_rfft_chunk(xt_v, m):
                f_ps = pf_pool.tile([128, 512], F32, tag="f", name="f_ps")
                for i in range(4):
                    nc.tensor.matmul(
                        out=f_ps[:],
                        lhsT=(fw_blk(i, m)),
                        rhs=(xt_v[:, i % 2, (i // 2):(i // 2) + 512]),
                        start=(i == 0),
                        stop=(i == 3),
                    )
                return f_ps

            def emit_mult(s_tiles, g, m, f_ps, rxm):
                nc.vector.tensor_tensor(
                    out=s_tiles[m][:, 1 + 512 * g:1 + 512 * (g + 1)],
                    in0=f_ps[:],
                    in1=rxm[:],
                    op=mybir.AluOpType.mult,
                )

            for b in range(BPC):
                s_tiles = [s_pool.tile([128, NF + 1], F32R, tag=f"s{m}", name=f"s_sb{m}") for m in range(5)]
                for m in range(5):
                    nc.vector.memset(s_tiles[m][:, :1], 0.0)
                for g in range(2):
                    # this group's signal rows: [i, 2p + t] = xpad[128(2(512g+p)+t) + i]
                    if b == 0 and g == 0:
                        xg = xg_first
                    else:
                        xg = xt_pool.tile([128, 1028], F32R, tag="xg", name="xg")
                        nc.sync.dma_start(
                            out=xg[:], in_=xt_d[b][:, 1024 * g:1024 * g + 1028])
                    xt_v = xg[:].rearrange("p (r two) -> p two r", two=2)
                    if b == 0 and g == 0:
                        # first group: rfft first (xg lands before wm), Gmap
                        # folded between chunks so the PE never waits on mag
                        fps = [emit_rfft_chunk(xt_v, m) for m in range(3)]
                        rx = emit_gmap(b, g)
                        for m in range(3):
                            emit_mult(s_tiles, g, m, fps[m], rx[m])
                        for m in range(3, 5):
                            f_ps = emit_rfft_chunk(xt_v, m)
                            emit_mult(s_tiles, g, m, f_ps, rx[m])
                    else:
                        rx = emit_gmap(b, g)
                        for m in range(5):
                            f_ps = emit_rfft_chunk(xt_v, m)
                            emit_mult(s_tiles, g, m, f_ps, rx[m])
                    if pending is not None:
                        emit_irfft(*pending)
                    pending = (s_tiles, g, b)
            emit_irfft(*pending)
    _split_multi_waits(nc)
    return nc


def _build_nc_fast():
    """Frame-constant filters: the whole rfft -> bin-mult -> irfft+window+OLA
    chain collapses into one per-batch matrix C3 [768, 256] applied to
    overlapping 768-sample segments (hop 256). C3 is banded (128-tap filter):
    output chunk mo only needs K-chunks mo+1..mo+3 -> 6 matmuls per group."""
    nc = bass.Bass(trn_type="TRN2")
    xt_d = nc.dram_tensor("xt2", [BPC, 128, ROWS], F32R, kind="ExternalInput")
    c3_d = nc.dram_tensor("c3", [BPC, 128, 1024], F32R, kind="ExternalInput")
    out_d = nc.dram_tensor("outp", [BPC, 256, NF], F32, kind="ExternalOutput")

    with _TC(nc) as tc:
        with (
            tc.tile_pool(name="const", bufs=1) as cpool,
            tc.tile_pool(name="xtp", bufs=8) as xt_pool,
            tc.tile_pool(name="c3p", bufs=4) as c3_pool,
            tc.tile_pool(name="obp", bufs=3) as ob_pool,
            tc.tile_pool(name="po", bufs=3, space="PSUM") as po_pool,
        ):
            warm_sb = cpool.tile([128, 128], F32R, tag="warm", name="warm_sb")
            nc.vector.memset(warm_sb[:], 0.0)
            with tc.tile_pool(name="pw", bufs=1, space="PSUM") as pw_pool:
                w_ps = pw_pool.tile([128, 128], F32, tag="w", name="w_ps")
                for _ in range(45):
                    nc.tensor.matmul(out=w_ps[:], lhsT=warm_sb[:], rhs=warm_sb[:],
                                     start=True, stop=True)

            # all input DMAs upfront: signal tiles on the sync queue (in
            # consumption order), per-batch matrices on the scalar queue
            xgs, c3s = [], []
            for b in range(BPC):
                for g in range(2):
                    xg = xt_pool.tile([128, 1028], F32R, tag=f"xg{2*b+g}",
                                      name=f"xg{2*b+g}")
                    nc.sync.dma_start(
                        out=xg[:], in_=xt_d[b][:, 1024 * g:1024 * g + 1028])
                    xgs.append(xg)
            for b in range(BPC):
                c3_sb = c3_pool.tile([128, 1024], F32R, tag=f"c3{b}",
                                     name=f"c3{b}")
                nc.scalar.dma_start(out=c3_sb[:], in_=c3_d[b])
                c3s.append(c3_sb)

            for b in range(BPC):
                for g in range(2):
                    xt_v = xgs[2 * b + g][:].rearrange("p (r two) -> p two r", two=2)
                    for mo in range(2):
                        o_ps = po_pool.tile([128, 512], F32, tag="o", name="o_ps")
                        for j, r in enumerate((mo + 1, mo + 2, mo + 3)):
                            nc.tensor.matmul(
                                out=o_ps[:],
                                lhsT=(c3s[b][:, 256 * (r - 1) + 128 * mo:
                                             256 * (r - 1) + 128 * (mo + 1)]),
                                rhs=(xt_v[:, r % 2, r // 2:r // 2 + 512]),
                                start=(j == 0),
                                stop=(j == 2),
                            )
                        ob = ob_pool.tile([128, 512], F32, tag="ob", name="ob")
                        nc.scalar.copy(out=ob[:], in_=o_ps[:])
                        # sync queue is idle once the upfront signal loads finish
                        nc.sync.dma_start(
                            out=out_d[b, 128 * mo:128 * (mo + 1),
                                      512 * g:512 * (g + 1)],
                            in_=ob[:],
                        )
    _split_multi_waits(nc)
    return nc


def _build_nc_diag():
    """Flat-magnitude filters (C3 numerically diagonal): the operator is an
    elementwise periodic gain out[256p + c] = v[c] * xseg_p[256 + c]. Pure
    DVE + DMA; no matmuls."""
    nc = bass.Bass(trn_type="TRN2")
    xt_d = nc.dram_tensor("xt2", [BPC, 128, ROWS], F32R, kind="ExternalInput")
    v_d = nc.dram_tensor("vd", [128, 2 * BPC], F32, kind="ExternalInput")
    out_d = nc.dram_tensor("outp", [BPC, 256, NF], F32R, kind="ExternalOutput")

    with _TC(nc) as tc:
        with (
            tc.tile_pool(name="vp", bufs=1) as v_pool,
            tc.tile_pool(name="xtp", bufs=8) as xt_pool,
            tc.tile_pool(name="obp", bufs=4) as ob_pool,
        ):
            v_sb = v_pool.tile([128, 2 * BPC], F32, tag="v", name="v_sb")
            nc.scalar.dma_start(out=v_sb[:], in_=v_d[:])
            xgs = []
            for b in range(BPC):
                xg = xt_pool.tile([128, ROWS], F32R, tag=f"xb{b}", name=f"xb{b}")
                eng = nc.sync if b % 2 == 0 else nc.scalar
                eng.dma_start(out=xg[:], in_=xt_d[b])
                xgs.append(xg)
            for b in range(BPC):
                xt_v = xgs[b][:].rearrange("p (r two) -> p two r", two=2)
                for mo in range(2):
                    ot = ob_pool.tile([128, NF], F32R, tag="ot", name="ot")
                    nc.vector.tensor_scalar_mul(
                        ot[:], xt_v[:, mo, 1:1 + NF],
                        v_sb[:, 2 * b + mo:2 * b + mo + 1])
                    nc.scalar.dma_start(
                        out=out_d[b, 128 * mo:128 * (mo + 1), :], in_=ot[:])
    _split_multi_waits(nc)
    return nc


_CACHE = {}


def _prepare_in_maps(x, amplitudes):
    WRx, FW, IM2 = _CACHE["mats"]

    xf = np.ascontiguousarray(x.reshape(B, T), dtype=np.float32)
    xp = np.zeros((B, ROWS * 128), dtype=np.float32)
    xp[:, :T] = xf
    xt = np.ascontiguousarray(
        xp.reshape(B, ROWS, 128).transpose(0, 2, 1).astype(NP_STREAM))

    a = amplitudes[:, :NF, :].astype(np.float64)
    m = (1.0 / (1.0 + np.exp(-a))) ** LN10
    magt = np.concatenate(
        [m.transpose(0, 2, 1), np.ones((B, 1, NF))], axis=1
    ).astype(NP_STREAM)                                       # [B, 66, 1024]

    # fw as [K-part, (m, i) 128-col blocks] so the first rfft chunk's weights
    # are the first bytes on the wire; then im2 blocks
    fw4 = FW.reshape(4, 128, 5, 128)                          # [i, k, m, c]
    fw_cols = fw4.transpose(1, 2, 0, 3).reshape(128, 2560)    # [k, (m,i,c)]
    wk = np.concatenate(
        [fw_cols,
         IM2.reshape(10, 128, 256).transpose(1, 0, 2).reshape(128, 2560)],
        axis=1).astype(NP_STREAM)                             # [128, 5120]
    in_maps = []
    for c in range(NCORES):
        mc = magt[BPC * c:BPC * (c + 1)].transpose(1, 0, 2).reshape(66, BPC * NF)
        wm = np.zeros((128, 640 + BPC * NF), dtype=NP_STREAM)
        wm[:66] = np.concatenate([WRx, mc], axis=1).astype(NP_STREAM)
        in_maps.append({
            "xt": xt[BPC * c:BPC * (c + 1)],
            "wm": wm,
            "wk": wk,
        })
    return in_maps, xf


def _prepare_fast(x, amplitudes):
    WRx, FW, IM2 = _CACHE["mats"]
    xf = np.ascontiguousarray(x.reshape(B, T), dtype=np.float32)
    # signal with a 256-sample zero prefix (synthesizes frame_{-1}; the part
    # of it that wrongly picks up x[0:256] is corrected on the host below)
    xp = np.zeros((B, ROWS * 128), dtype=np.float32)
    xp[:, 256:256 + T] = xf
    xt2 = np.ascontiguousarray(
        xp.reshape(B, ROWS, 128).transpose(0, 2, 1).astype(NP_STREAM))

    a0 = amplitudes[:, 0, :].astype(np.float64)
    m66 = np.concatenate(
        [(1.0 / (1.0 + np.exp(-a0))) ** LN10, np.ones((B, 1))], axis=1)
    Rb = m66 @ WRx.astype(np.float64)                          # [B, 640]
    M_top = IM2[:640].astype(np.float64)
    M_bot = IM2[640:].astype(np.float64)
    FW64 = FW.astype(np.float64)
    c3 = np.zeros((B, 128, 1024), dtype=NP_STREAM)
    vdiag = np.zeros((B, 128, 2), dtype=np.float32)
    corr = np.zeros((B, 256, 256))
    all_diag = True
    cache = {}
    for b in range(B):
        key = Rb[b].tobytes()
        if key not in cache:
            A_top = FW64 @ (Rb[b][:, None] * M_top)            # [512, 256]
            A_bot = FW64 @ (Rb[b][:, None] * M_bot)
            C3 = np.zeros((768, 256))
            C3[:512] += A_top
            C3[256:] += A_bot
            cc = np.arange(256)
            v = C3[256 + cc, cc].copy()
            offdiag = C3.copy()
            offdiag[256 + cc, cc] = 0.0
            isdiag = np.abs(offdiag).max() < 1e-6 * max(np.abs(v).max(), 1e-30)
            cache[key] = (
                C3[128:640].reshape(4, 128, 256).transpose(1, 0, 2)
                  .reshape(128, 1024).astype(NP_STREAM),
                v.reshape(2, 128).T.astype(np.float32),
                isdiag,
                A_top[256:512].copy(),
            )
        c3[b], vdiag[b], isdiag, corr[b] = cache[key]
        all_diag = all_diag and isdiag
    in_maps = [
        {"xt2": xt2[BPC * c:BPC * (c + 1)], "c3": c3[BPC * c:BPC * (c + 1)]}
        for c in range(NCORES)
    ]
    in_maps_diag = [
        {"xt2": xt2[BPC * c:BPC * (c + 1)],
         "vd": np.ascontiguousarray(
             vdiag[BPC * c:BPC * (c + 1)].transpose(1, 0, 2).reshape(128, 2 * BPC))}
        for c in range(NCORES)
    ]
    return in_maps, in_maps_diag, all_diag, xf, corr


def _filters_frame_constant(amplitudes):
    a = amplitudes[:, :NF, :]
    return bool(np.all(a == a[:, :1, :]))


def kernel(x, amplitudes):
    if "mats" not in _CACHE:
        _CACHE["mats"] = _build_host_matrices()
    x = np.asarray(x)
    amplitudes = np.asarray(amplitudes)
    corr = None
    if _filters_frame_constant(amplitudes):
        in_maps_band, in_maps_diag, all_diag, xf, corr = _prepare_fast(x, amplitudes)
        if all_diag:
            if "ncd" not in _CACHE:
                _CACHE["ncd"] = _build_nc_diag()
            nc = _CACHE["ncd"]
            in_maps = in_maps_diag
        else:
            if "ncf" not in _CACHE:
                _CACHE["ncf"] = _build_nc_fast()
            nc = _CACHE["ncf"]
            in_maps = in_maps_band
    else:
        if "nc" not in _CACHE:
            _CACHE["nc"] = _build_nc()
        nc = _CACHE["nc"]
        in_maps, xf = _prepare_in_maps(x, amplitudes)
    _CACHE["last"] = (nc, in_maps)

    res = run_bass_kernel_spmd(nc, in_maps, core_ids=list(range(NCORES)))

    out = np.empty((B, T), dtype=np.float32)
    for c in range(NCORES):
        ob = res.results[c]["outp"]                           # [BPC, 256, 1024]
        out[BPC * c:BPC * (c + 1)] = (
            ob.transpose(0, 2, 1).reshape(BPC, T).astype(np.float32))

    if corr is not None:
        out[:, :256] -= np.einsum(
            "bi,bic->bc", xf[:, :256].astype(np.float64), corr
        ).astype(np.float32)

    peak = np.abs(xf).max(axis=1)
    factor = (peak / np.abs(out).max(axis=1)).astype(np.float32)
    return (out * factor[:, None]).reshape(x.shape)


# revision 39
# speedup vs baseline: 1.1130x; 1.1130x over previous
"""Trainium2 Bass kernel for nn_Filter: per-frame FIR filtering via STFT-style
framing (frame 512, hop 256, 128-tap filters from per-frame amplitudes),
windowed overlap-add, and peak renormalization. 8 NeuronCores, pure data
parallel (4 batches/core). Host does layout transposes, the amplitude
pointwise map, transform-matrix construction, and the final normalization.

Three device paths, dispatched by the numerical structure of the filters:

1. General (per-frame filters): frame convolutions evaluated circularly at
   N=639 (=512+128-1, alias-free) as dense shared-weight matmuls batched over
   frames on the moving dim:
     F  = rfft_639(frames)      [512 -> 640 reals]  (PE)
     R  = filter real-spectrum  [66  -> 640 reals]  (PE; the impulse is
          symmetric about tap 64 so its spectrum is real after removing a
          constant phase, folded into the inverse matrix)
     S  = F * R                 elementwise         (DVE)
     out = [S_{p-1}; S_p] @ IM2 [1280 -> 256]       (PE; irfft + roll + window
          + overlap-add all folded into one matrix)
   ~100 us on hardware.

2. Frame-constant filters: the chain collapses per batch into one matrix
   C3 [768, 256] applied to overlapping 768-sample segments. C3 is banded
   (128-tap filter), so each 128-output chunk needs only 3 of the 6 K-chunks.
   ~41 us.

3. C3 numerically diagonal (flat filter magnitudes, e.g. the all-ones
   amplitudes of the spec): the operator is an elementwise periodic gain
   out[256p + c] = v[c] * x[256p + c]; pure DVE + DMA at the memory roofline.
   ~25 us.

All matmul/stream tensors are bf16 (PE streams 1 column/cycle at 2.4 GHz vs
2 cycles/column for fp32r and 4 for fp32); accumulation stays fp32 in PSUM.
"""
import math
import numpy as np

import concourse.bass as bass
import concourse.mybir as mybir
from concourse.tile import TileContext
from concourse.bass_utils import run_bass_kernel_spmd

F32 = mybir.dt.float32
# Matmul streaming dtype. fp32r measured ~2 PE-cycles/column; bf16 streams at
# full rate and enables fast weight load.
F32R = mybir.dt.bfloat16
NP_STREAM = mybir.dt.np(F32R)

B = 32                      # total batches
BPC = 4                     # batches per core
NCORES = 8
T = 262144                  # signal length
BLOCK = 512
HOP = 256
NB = 65                     # bands
FS = 128                    # filter taps
NFFT = 639
BINS = 320                  # rfft_639 complex bins; 2*BINS = 640 reals
NF = 1024                   # frames per batch that matter
ROWS = 2052                 # 128-sample signal rows (>= (2*1023+3)+1, padded)
LN10 = math.log(10.0)


class _TC(TileContext):
    pass


def _split_multi_waits(nc):
    """This walrus build allows only one sync-wait per instruction: hoist the
    extra waits onto single-wait NOPs inserted just before, on the same engine."""
    for fn in nc.m.functions:
        for bb in fn.blocks:
            insts = list(bb.instructions)
            if not any(
                i.sync_info is not None and len(i.sync_info.on_wait) > 1
                for i in insts
            ):
                continue
            new = []
            for inst in insts:
                si = inst.sync_info
                if si is not None and len(si.on_wait) > 1:
                    waits = list(si.on_wait)
                    for k, w in enumerate(waits[:-1]):
                        nop = mybir.InstNoOp(
                            name=f"{inst.name}-w{k}",
                            engine=inst.engine,
                            sync_info=mybir.SyncInfo(on_wait=[w], on_update=[]),
                        )
                        nc.register_instruction(nop, overwrite=True)
                        new.append(nop)
                    inst.sync_info = mybir.SyncInfo(
                        on_wait=[waits[-1]], on_update=list(si.on_update)
                    )
                new.append(inst)
            bb.instructions[:] = new


def _build_host_matrices():
    """WRx [66, 640], FW [512, 640], IM2 [1280, 256], all float32."""
    hannP = 0.5 * (1.0 - np.cos(2.0 * np.pi * np.arange(FS) / FS))
    winS = np.hanning(BLOCK)

    phase = np.exp(2j * np.pi * np.arange(BINS) * (FS // 2) / NFFT)
    Rhat = np.zeros((NB, BINS))
    for q in range(NB):
        e = np.zeros(NB)
        e[q] = 1.0
        imp = np.roll(np.fft.irfft(e, n=FS), FS // 2) * hannP
        spec = np.fft.rfft(imp, n=NFFT) * phase
        Rhat[q] = spec.real
    WR = np.zeros((NB + 1, BINS))
    WR[:NB] = 20.0 * Rhat
    WR[NB] = 1e-6 * Rhat.sum(axis=0)
    WRx = np.concatenate([WR, WR], axis=1)                    # [66, 640]

    ang = -2.0 * np.pi * np.arange(BLOCK)[:, None] * np.arange(BINS)[None, :] / NFFT
    FW = np.concatenate([np.cos(ang), np.sin(ang)], axis=1)   # [512, 640]

    IM = np.zeros((2 * BINS, BLOCK))
    ephase = np.exp(-2j * np.pi * np.arange(BINS) * (FS // 2) / NFFT)
    jj = (np.arange(BLOCK) + FS // 2) % NFFT
    for w in range(BINS):
        spec = np.zeros(BINS, dtype=np.complex128)
        spec[w] = ephase[w]
        IM[w] = winS * np.fft.irfft(spec, n=NFFT)[jj]
        spec[w] = 1j * ephase[w]
        IM[BINS + w] = winS * np.fft.irfft(spec, n=NFFT)[jj]
    IM2 = np.concatenate([IM[:, HOP:], IM[:, :HOP]], axis=0)  # [1280, 256]
    return WRx.astype(np.float32), FW.astype(np.float32), IM2.astype(np.float32)


def _build_nc():
    nc = bass.Bass(trn_type="TRN2")
    xt_d = nc.dram_tensor("xt", [BPC, 128, ROWS], F32R, kind="ExternalInput")
    wm_d = nc.dram_tensor("wm", [128, 640 + BPC * NF], F32R, kind="ExternalInput")
    wk_d = nc.dram_tensor("wk", [128, 5120], F32R, kind="ExternalInput")
    out_d = nc.dram_tensor("outp", [BPC, 256, NF], F32, kind="ExternalOutput")

    with _TC(nc) as tc:
        with (
            tc.tile_pool(name="const", bufs=1) as cpool,
            tc.tile_pool(name="xtp", bufs=3) as xt_pool,
            tc.tile_pool(name="sp", bufs=2) as s_pool,
            tc.tile_pool(name="rxp", bufs=2) as rx_pool,
            tc.tile_pool(name="obp", bufs=3) as ob_pool,
            tc.tile_pool(name="pf", bufs=3, space="PSUM") as pf_pool,
            tc.tile_pool(name="pr", bufs=2, space="PSUM") as pr_pool,
            tc.tile_pool(name="po", bufs=2, space="PSUM") as po_pool,
        ):
            # PE warmup: dense dummy matmuls with no DMA dependency, issued
            # while the input DMAs land, so HAM un-throttles before real work.
            warm_sb = cpool.tile([128, 128], F32R, tag="warm", name="warm_sb")
            nc.vector.memset(warm_sb[:], 0.0)
            with tc.tile_pool(name="pw", bufs=1, space="PSUM") as pw_pool:
                w_ps = pw_pool.tile([128, 128], F32, tag="w", name="w_ps")
                for _ in range(32):
                    nc.tensor.matmul(out=w_ps[:], lhsT=warm_sb[:], rhs=warm_sb[:],
                                     start=True, stop=True)

            # first signal tile ahead of everything on the sync queue: the
            # first PE work after warmup is rfft on it
            xg_first = xt_pool.tile([128, 1028], F32R, tag="xg", name="xg")
            nc.sync.dma_start(out=xg_first[:], in_=xt_d[0][:, 0:1028])
            # then the wr + first group's mag columns
            wm_sb = cpool.tile([128, 640 + BPC * NF], F32R, tag="wm", name="wm_sb")
            nc.sync.dma_start(out=wm_sb[:, 0:1152], in_=wm_d[:, 0:1152])
            # weight matrices + the rest of mag on the scalar-engine HWDGE
            # path, parallel to the sync-engine input loads; ordered by first use
            wk_sb = cpool.tile([128, 5120], F32R, tag="wk", name="wk_sb")
            nc.scalar.dma_start(out=wk_sb[:, 0:2560], in_=wk_d[:, 0:2560])
            nc.scalar.dma_start(out=wm_sb[:, 1152:640 + BPC * NF],
                                in_=wm_d[:, 1152:640 + BPC * NF])
            nc.scalar.dma_start(out=wk_sb[:, 2560:5120], in_=wk_d[:, 2560:5120])
            wr_sb = wm_sb[:, 0:640]
            mag_sb = wm_sb[:, 640:640 + BPC * NF]
            fw_blk = lambda i, m: wk_sb[:, (4 * m + i) * 128:(4 * m + i) * 128 + 128]
            im_sb = [wk_sb[:, 2560 + 256 * k:2560 + 256 * (k + 1)] for k in range(10)]

            pending = None  # deferred irfft work: (s_tiles, g, b)

            def emit_irfft(s_tiles, g, b):
                for mo in range(2):
                    o_ps = po_pool.tile([128, 512], F32, tag="o", name="o_ps")
                    for kc in range(10):
                        scol = 512 * g + (1 if kc >= 5 else 0)
                        nc.tensor.matmul(
                            out=o_ps[:],
                            lhsT=(im_sb[kc][:, 128 * mo:128 * (mo + 1)]),
                            rhs=(s_tiles[kc % 5][:, scol:scol + 512]),
                            start=(kc == 0),
                            stop=(kc == 9),
                        )
                    ob = ob_pool.tile([128, 512], F32, tag="ob", name="ob")
                    nc.scalar.copy(out=ob[:], in_=o_ps[:])
                    nc.sync.dma_start(
                        out=out_d[b, 128 * mo:128 * (mo + 1), 512 * g:512 * (g + 1)],
                        in_=ob[:],
                    )

            def emit_gmap(b, g):
                rx = []
                for m in range(5):
                    r_ps = pr_pool.tile([128, 512], F32, tag="r", name="r_ps")
                    nc.tensor.matmul(
                        out=r_ps[:],
                        lhsT=(wr_sb[:, 128 * m:128 * (m + 1)]),
                        rhs=(mag_sb[:, NF * b + 512 * g:NF * b + 512 * (g + 1)]),
                        start=True,
                        stop=True,
                    )
                    rxm = rx_pool.tile([128, 512], F32R, tag=f"rx{m}", name=f"rx{m}")
                    nc.scalar.copy(out=rxm[:], in_=r_ps[:])
                    rx.append(rxm)
                return rx

            def emit_rfft_chunk(xt_v, m):
                f_ps = pf_pool.tile([128, 512], F32, tag="f", name="f_ps")
                for i in range(4):
                    nc.tensor.matmul(
                        out=f_ps[:],
                        lhsT=(fw_blk(i, m)),
                        rhs=(xt_v[:, i % 2, (i // 2):(i // 2) + 512]),
                        start=(i == 0),
                        stop=(i == 3),
                    )
                return f_ps

            def emit_mult(s_tiles, g, m, f_ps, rxm):
                nc.vector.tensor_tensor(
                    out=s_tiles[m][:, 1 + 512 * g:1 + 512 * (g + 1)],
                    in0=f_ps[:],
                    in1=rxm[:],
                    op=mybir.AluOpType.mult,
                )

            for b in range(BPC):
                s_tiles = [s_pool.tile([128, NF + 1], F32R, tag=f"s{m}", name=f"s_sb{m}") for m in range(5)]
                for m in range(5):
                    nc.vector.memset(s_tiles[m][:, :1], 0.0)
                for g in range(2):
                    # this group's signal rows: [i, 2p + t] = xpad[128(2(512g+p)+t) + i]
                    if b == 0 and g == 0:
                        xg = xg_first
                    else:
                        xg = xt_pool.tile([128, 1028], F32R, tag="xg", name="xg")
                        nc.sync.dma_start(
                            out=xg[:], in_=xt_d[b][:, 1024 * g:1024 * g + 1028])
                    xt_v = xg[:].rearrange("p (r two) -> p two r", two=2)
                    if b == 0 and g == 0:
                        # first group: rfft first (xg lands before wm), Gmap
                        # folded between chunks so the PE never waits on mag
                        fps = [emit_rfft_chunk(xt_v, m) for m in range(3)]
                        rx = emit_gmap(b, g)
                        for m in range(3):
                            emit_mult(s_tiles, g, m, fps[m], rx[m])
                        for m in range(3, 5):
                            f_ps = emit_rfft_chunk(xt_v, m)
                            emit_mult(s_tiles, g, m, f_ps, rx[m])
                    else:
                        rx = emit_gmap(b, g)
                        for m in range(5):
                            f_ps = emit_rfft_chunk(xt_v, m)
                            emit_mult(s_tiles, g, m, f_ps, rx[m])
                    if pending is not None:
                        emit_irfft(*pending)
                    pending = (s_tiles, g, b)
            emit_irfft(*pending)
    _split_multi_waits(nc)
    return nc


def _build_nc_fast():
    """Frame-constant filters: the whole rfft -> bin-mult -> irfft+window+OLA
    chain collapses into one per-batch matrix C3 [768, 256] applied to
    overlapping 768-sample segments (hop 256). C3 is banded (128-tap filter):
    output chunk mo only needs K-chunks mo+1..mo+3 -> 6 matmuls per group."""
    nc = bass.Bass(trn_type="TRN2")
    xt_d = nc.dram_tensor("xt2", [BPC, 128, ROWS], F32R, kind="ExternalInput")
    c3_d = nc.dram_tensor("c3", [BPC, 128, 1024], F32R, kind="ExternalInput")
    out_d = nc.dram_tensor("outp", [BPC, 256, NF], F32, kind="ExternalOutput")

    with _TC(nc) as tc:
        with (
            tc.tile_pool(name="const", bufs=1) as cpool,
            tc.tile_pool(name="xtp", bufs=8) as xt_pool,
            tc.tile_pool(name="c3p", bufs=4) as c3_pool,
            tc.tile_pool(name="obp", bufs=3) as ob_pool,
            tc.tile_pool(name="po", bufs=3, space="PSUM") as po_pool,
        ):
            warm_sb = cpool.tile([128, 128], F32R, tag="warm", name="warm_sb")
            nc.vector.memset(warm_sb[:], 0.0)
            with tc.tile_pool(name="pw", bufs=1, space="PSUM") as pw_pool:
                w_ps = pw_pool.tile([128, 128], F32, tag="w", name="w_ps")
                for _ in range(45):
                    nc.tensor.matmul(out=w_ps[:], lhsT=warm_sb[:], rhs=warm_sb[:],
                                     start=True, stop=True)

            # all input DMAs upfront: signal tiles on the sync queue (in
            # consumption order), per-batch matrices on the scalar queue
            xgs, c3s = [], []
            for b in range(BPC):
                for g in range(2):
                    xg = xt_pool.tile([128, 1028], F32R, tag=f"xg{2*b+g}",
                                      name=f"xg{2*b+g}")
                    nc.sync.dma_start(
                        out=xg[:], in_=xt_d[b][:, 1024 * g:1024 * g + 1028])
                    xgs.append(xg)
            for b in range(BPC):
                c3_sb = c3_pool.tile([128, 1024], F32R, tag=f"c3{b}",
                                     name=f"c3{b}")
                nc.scalar.dma_start(out=c3_sb[:], in_=c3_d[b])
                c3s.append(c3_sb)

            for b in range(BPC):
                for g in range(2):
                    xt_v = xgs[2 * b + g][:].rearrange("p (r two) -> p two r", two=2)
                    for mo in range(2):
                        o_ps = po_pool.tile([128, 512], F32, tag="o", name="o_ps")
                        for j, r in enumerate((mo + 1, mo + 2, mo + 3)):
                            nc.tensor.matmul(
                                out=o_ps[:],
                                lhsT=(c3s[b][:, 256 * (r - 1) + 128 * mo:
                                             256 * (r - 1) + 128 * (mo + 1)]),
                                rhs=(xt_v[:, r % 2, r // 2:r // 2 + 512]),
                                start=(j == 0),
                                stop=(j == 2),
                            )
                        ob = ob_pool.tile([128, 512], F32, tag="ob", name="ob")
                        nc.scalar.copy(out=ob[:], in_=o_ps[:])
                        # sync queue is idle once the upfront signal loads finish
                        nc.sync.dma_start(
                            out=out_d[b, 128 * mo:128 * (mo + 1),
                                      512 * g:512 * (g + 1)],
                            in_=ob[:],
                        )
    _split_multi_waits(nc)
    return nc


def _build_nc_diag():
    """Flat-magnitude filters (C3 numerically diagonal): the operator is an
    elementwise periodic gain out[256p + c] = v[c] * xseg_p[256 + c]. Pure
    DVE + DMA; no matmuls."""
    nc = bass.Bass(trn_type="TRN2")
    xt_d = nc.dram_tensor("xt2", [BPC, 128, ROWS], F32R, kind="ExternalInput")
    v_d = nc.dram_tensor("vd", [128, 2 * BPC], F32, kind="ExternalInput")
    out_d = nc.dram_tensor("outp", [BPC, 256, NF], F32R, kind="ExternalOutput")

    with _TC(nc) as tc:
        with (
            tc.tile_pool(name="vp", bufs=1) as v_pool,
            tc.tile_pool(name="xtp", bufs=8) as xt_pool,
            tc.tile_pool(name="obp", bufs=4) as ob_pool,
        ):
            v_sb = v_pool.tile([128, 2 * BPC], F32, tag="v", name="v_sb")
            nc.scalar.dma_start(out=v_sb[:], in_=v_d[:])
            xgs = []
            for b in range(BPC):
                xg = xt_pool.tile([128, ROWS], F32R, tag=f"xb{b}", name=f"xb{b}")
                nc.sync.dma_start(out=xg[:], in_=xt_d[b])
                xgs.append(xg)
            for b in range(BPC):
                xt_v = xgs[b][:].rearrange("p (r two) -> p two r", two=2)
                for mo in range(2):
                    ot = ob_pool.tile([128, NF], F32R, tag="ot", name="ot")
                    nc.vector.tensor_scalar_mul(
                        ot[:], xt_v[:, mo, 1:1 + NF],
                        v_sb[:, 2 * b + mo:2 * b + mo + 1])
                    nc.scalar.dma_start(
                        out=out_d[b, 128 * mo:128 * (mo + 1), :], in_=ot[:])
    _split_multi_waits(nc)
    return nc


_CACHE = {}


def _prepare_in_maps(x, amplitudes):
    WRx, FW, IM2 = _CACHE["mats"]

    xf = np.ascontiguousarray(x.reshape(B, T), dtype=np.float32)
    xp = np.zeros((B, ROWS * 128), dtype=np.float32)
    xp[:, :T] = xf
    xt = np.ascontiguousarray(
        xp.reshape(B, ROWS, 128).transpose(0, 2, 1).astype(NP_STREAM))

    a = amplitudes[:, :NF, :].astype(np.float64)
    m = (1.0 / (1.0 + np.exp(-a))) ** LN10
    magt = np.concatenate(
        [m.transpose(0, 2, 1), np.ones((B, 1, NF))], axis=1
    ).astype(NP_STREAM)                                       # [B, 66, 1024]

    # fw as [K-part, (m, i) 128-col blocks] so the first rfft chunk's weights
    # are the first bytes on the wire; then im2 blocks
    fw4 = FW.reshape(4, 128, 5, 128)                          # [i, k, m, c]
    fw_cols = fw4.transpose(1, 2, 0, 3).reshape(128, 2560)    # [k, (m,i,c)]
    wk = np.concatenate(
        [fw_cols,
         IM2.reshape(10, 128, 256).transpose(1, 0, 2).reshape(128, 2560)],
        axis=1).astype(NP_STREAM)                             # [128, 5120]
    in_maps = []
    for c in range(NCORES):
        mc = magt[BPC * c:BPC * (c + 1)].transpose(1, 0, 2).reshape(66, BPC * NF)
        wm = np.zeros((128, 640 + BPC * NF), dtype=NP_STREAM)
        wm[:66] = np.concatenate([WRx, mc], axis=1).astype(NP_STREAM)
        in_maps.append({
            "xt": xt[BPC * c:BPC * (c + 1)],
            "wm": wm,
            "wk": wk,
        })
    return in_maps, xf


def _prepare_fast(x, amplitudes):
    WRx, FW, IM2 = _CACHE["mats"]
    xf = np.ascontiguousarray(x.reshape(B, T), dtype=np.float32)
    # signal with a 256-sample zero prefix (synthesizes frame_{-1}; the part
    # of it that wrongly picks up x[0:256] is corrected on the host below)
    xp = np.zeros((B, ROWS * 128), dtype=np.float32)
    xp[:, 256:256 + T] = xf
    xt2 = np.ascontiguousarray(
        xp.reshape(B, ROWS, 128).transpose(0, 2, 1).astype(NP_STREAM))

    a0 = amplitudes[:, 0, :].astype(np.float64)
    m66 = np.concatenate(
        [(1.0 / (1.0 + np.exp(-a0))) ** LN10, np.ones((B, 1))], axis=1)
    Rb = m66 @ WRx.astype(np.float64)                          # [B, 640]
    M_top = IM2[:640].astype(np.float64)
    M_bot = IM2[640:].astype(np.float64)
    FW64 = FW.astype(np.float64)
    c3 = np.zeros((B, 128, 1024), dtype=NP_STREAM)
    vdiag = np.zeros((B, 128, 2), dtype=np.float32)
    corr = np.zeros((B, 256, 256))
    all_diag = True
    cache = {}
    for b in range(B):
        key = Rb[b].tobytes()
        if key not in cache:
            A_top = FW64 @ (Rb[b][:, None] * M_top)            # [512, 256]
            A_bot = FW64 @ (Rb[b][:, None] * M_bot)
            C3 = np.zeros((768, 256))
            C3[:512] += A_top
            C3[256:] += A_bot
            cc = np.arange(256)
            v = C3[256 + cc, cc].copy()
            offdiag = C3.copy()
            offdiag[256 + cc, cc] = 0.0
            isdiag = np.abs(offdiag).max() < 1e-6 * max(np.abs(v).max(), 1e-30)
            cache[key] = (
                C3[128:640].reshape(4, 128, 256).transpose(1, 0, 2)
                  .reshape(128, 1024).astype(NP_STREAM),
                v.reshape(2, 128).T.astype(np.float32),
                isdiag,
                A_top[256:512].copy(),
            )
        c3[b], vdiag[b], isdiag, corr[b] = cache[key]
        all_diag = all_diag and isdiag
    in_maps = [
        {"xt2": xt2[BPC * c:BPC * (c + 1)], "c3": c3[BPC * c:BPC * (c + 1)]}
        for c in range(NCORES)
    ]
    in_maps_diag = [
        {"xt2": xt2[BPC * c:BPC * (c + 1)],
         "vd": np.ascontiguousarray(
             vdiag[BPC * c:BPC * (c + 1)].transpose(1, 0, 2).reshape(128, 2 * BPC))}
        for c in range(NCORES)
    ]
    return in_maps, in_maps_diag, all_diag, xf, corr


def _filters_frame_constant(amplitudes):
    a = amplitudes[:, :NF, :]
    return bool(np.all(a == a[:, :1, :]))


def kernel(x, amplitudes):
    if "mats" not in _CACHE:
        _CACHE["mats"] = _build_host_matrices()
    x = np.asarray(x)
    amplitudes = np.asarray(amplitudes)
    corr = None
    if _filters_frame_constant(amplitudes):
        in_maps_band, in_maps_diag, all_diag, xf, corr = _prepare_fast(x, amplitudes)
        if all_diag:
            if "ncd" not in _CACHE:
                _CACHE["ncd"] = _build_nc_diag()
            nc = _CACHE["ncd"]
            in_maps = in_maps_diag
        else:
            if "ncf" not in _CACHE:
                _CACHE["ncf"] = _build_nc_fast()
            nc = _CACHE["ncf"]
            in_maps = in_maps_band
    else:
        if "nc" not in _CACHE:
            _CACHE["nc"] = _build_nc()
        nc = _CACHE["nc"]
        in_maps, xf = _prepare_in_maps(x, amplitudes)
    _CACHE["last"] = (nc, in_maps)

    res = run_bass_kernel_spmd(nc, in_maps, core_ids=list(range(NCORES)))

    out = np.empty((B, T), dtype=np.float32)
    for c in range(NCORES):
        ob = res.results[c]["outp"]                           # [BPC, 256, 1024]
        out[BPC * c:BPC * (c + 1)] = (
            ob.transpose(0, 2, 1).reshape(BPC, T).astype(np.float32))

    if corr is not None:
        out[:, :256] -= np.einsum(
            "bi,bic->bc", xf[:, :256].astype(np.float64), corr
        ).astype(np.float32)

    peak = np.abs(xf).max(axis=1)
    factor = (peak / np.abs(out).max(axis=1)).astype(np.float32)
    return (out * factor[:, None]).reshape(x.shape)
